# revision 20
# baseline (speedup 1.0000x reference)
"""Distributed Hamming k-means (LSH moe_routing) Bass kernel for 8 TRN2 cores.

Algorithm notes (exactness):
- bits b = (x@P.T + bias > 0); work in u = 2b-1 (+-1) space.
- Hamming dist d(p,k) = (32 - u_p . w_k)/2, so argmin_k d = argmax_k dot.
- Assign matmul computes combined = 1024*dot + 32768 + (511-k) using two
  extra all-ones contraction rows (W rows store +-1024 and the encode rows);
  every term is an exactly-representable integer, so argmax(combined)
  reproduces argmin-with-first-index-tie-break exactly.
- Inverted one-hot h = Sign(max - combined) in {0(argmax), +1}; stats matmul
  S_h = u_aug^T @ h, true stats = T - sum_cores(S_h) with T = u_aug col sums
  (T is AllReduced once; per-iter stats use AllGather + on-device sum).
- Vote: new w = +1024 iff votesum > 0; empty clusters keep old w.

Perf structure:
- Assign matmuls are 2-way row-tiled (contraction 34 <= 64): tile A uses PE
  rows 0-63 with lhsT/rhs at partition 0, tile B uses rows 64-127 with
  SBUF copies of uT_aug/W at partition 64 (replicated via SBUF->SBUF DMA).
- PSUM: 3x [128,1024] assign pairs + 2x shared misc/stats bank = 8 banks.
"""

import numpy as np

L, E, BITS, K = 131072, 256, 32, 512
N_CORES, N_ITER = 8, 10
LP = L // N_CORES            # 16384 points per core
P = 128                      # partitions / tile height
PAIR_F = 1024                # assign psum free size (2 banks, 2 tiles)
CR = BITS + 2                # assign contraction rows (32 bits + 2 encode)

_ENC = 511 - np.arange(K)    # tie-break code per cluster
_WROW_HI = 32768.0 + 256.0 * (_ENC >> 8)   # bf16-exact
_WROW_LO = (_ENC & 255).astype(np.float64)  # bf16-exact


def build_kernel(nt):
    """Build the SPMD Bass program. nt = number of 128-point tiles per core."""
    import concourse.bacc as bacc
    import concourse.mybir as mybir
    import concourse.tile as tile

    lp = nt * P
    f32 = mybir.dt.float32
    bf16 = mybir.dt.bfloat16
    i32 = mybir.dt.int32
    u8 = mybir.dt.uint8
    AX = mybir.AxisListType
    OP = mybir.AluOpType
    ACT = mybir.ActivationFunctionType

    nc = bacc.Bacc(None, target_bir_lowering=False, debug=False)

    xt = nc.dram_tensor("xt", [E, lp], f32, kind="ExternalInput")
    xinit_t = nc.dram_tensor("xinit_t", [E, K], f32, kind="ExternalInput")
    planes_t = nc.dram_tensor("planes_t", [E + 1, BITS], f32, kind="ExternalInput")
    wrows = nc.dram_tensor("wrows", [2, K], bf16, kind="ExternalInput")
    ident_b = nc.dram_tensor("ident_b", [P, P], bf16, kind="ExternalInput")
    ident_f = nc.dram_tensor("ident_f", [P, P], f32, kind="ExternalInput")
    labels_d = nc.dram_tensor("labels", [nt, P], i32, kind="ExternalOutput")
    counts_d = nc.dram_tensor("counts", [1, K], i32, kind="ExternalOutput")

    rg = [list(range(N_CORES))]

    with tile.TileContext(nc) as tc:
        with (
            tc.tile_pool(name="const", bufs=1) as constp,
            tc.tile_pool(name="big", bufs=1) as bigp,
            tc.tile_pool(name="work", bufs=3) as workp,
            tc.tile_pool(name="htile", bufs=4) as hp,
            tc.tile_pool(name="psA", bufs=6, space="PSUM") as psA,
            tc.tile_pool(name="psS", bufs=1, space="PSUM") as psS,
            tc.tile_pool(name="dram", bufs=2, space="DRAM") as dramp,
        ):
            # ---------------- static tiles ----------------
            # uT_aug rows 0-31: u codes (transposed); rows 32,33: ones
            uT_aug = bigp.tile([CR, lp], bf16)
            # replica at partition 64 for row-tiled matmul B
            uT_aug2 = bigp.tile([64 + CR, lp], bf16)
            # u_norm: per tile [128, 33] = [u | 1] blocks, 33-strided
            u_norm = bigp.tile([P, nt * 33], bf16)
            ident_sb = constp.tile([P, P], bf16)
            identf_sb = constp.tile([P, P], f32)
            pt0 = constp.tile([P, BITS], f32)           # planes_t rows 0:128
            pt1 = constp.tile([P, BITS], f32)           # planes_t rows 128:256
            ptb = constp.tile([1, BITS], f32)           # bias row
            ones_row = constp.tile([1, K], f32)         # bias-row rhs
            T_sb = constp.tile([33, 1], f32)            # global u_aug col totals
            mxall = bigp.tile([P, nt], f32)             # per-tile max values

            nc.sync.dma_start(ident_sb[:], ident_b[:])
            nc.sync.dma_start(identf_sb[:], ident_f[:])
            nc.sync.dma_start(pt0[:], planes_t[0:P, :])
            nc.sync.dma_start(pt1[:], planes_t[P:2 * P, :])
            nc.sync.dma_start(ptb[:], planes_t[2 * P:2 * P + 1, :])
            nc.vector.memset(ones_row[:], 1.0)
            nc.vector.memset(uT_aug[BITS:CR, :], 1.0)
            nc.vector.memset(u_norm[:], 1.0)

            # ---------------- phase H: hash ----------------
            def hash_chunk(dst_ap, src_cols_a, src_cols_b, width):
                """proj^T for `width` points -> Sign -> dst_ap (bf16 +-1)."""
                pp = psS.tile([BITS, width], f32, tag="misc")
                nc.tensor.matmul(pp[:], pt0[:], src_cols_a, start=True,
                                 stop=False)
                nc.tensor.matmul(pp[:], pt1[:], src_cols_b,
                                 start=False, stop=False)
                nc.tensor.matmul(pp[:], ptb[:],
                                 ones_row[:, 0:width], start=False, stop=True)
                nc.scalar.activation(dst_ap, pp[:], ACT.Sign)

            CH = 512
            for j in range(0, lp, CH):
                xa = workp.tile([P, CH], f32, tag="xa")
                xb = workp.tile([P, CH], f32, tag="xb")
                nc.sync.dma_start(xa[:], xt[0:P, j:j + CH])
                nc.sync.dma_start(xb[:], xt[P:2 * P, j:j + CH])
                hash_chunk(uT_aug[0:BITS, j:j + CH], xa[:], xb[:], CH)

            # replicate codes+ones to partition 64 (SBUF->SBUF DMA)
            nc.sync.dma_start(uT_aug2[64:64 + CR, :], uT_aug[:])

            # C0 from the 512 init points (replicated input); stored +-1024
            W_sgn = workp.tile([BITS, K], bf16, tag="Wsgn")
            xa = workp.tile([P, K], f32, tag="xa")
            xb = workp.tile([P, K], f32, tag="xb")
            nc.sync.dma_start(xa[:], xinit_t[0:P, :])
            nc.sync.dma_start(xb[:], xinit_t[P:2 * P, :])
            hash_chunk(W_sgn[:], xa[:], xb[:], K)
            W_cur = workp.tile([CR, K], bf16, tag="W")
            nc.vector.tensor_scalar(W_cur[0:BITS, :], W_sgn[:], 1024.0, None,
                                    op0=OP.mult)
            nc.sync.dma_start(W_cur[BITS:CR, :], wrows[:])

            # u_norm via PE transposes (4 tiles share one psum buffer)
            for t4 in range(0, nt, 4):
                ptr = psS.tile([P, P], bf16, tag="misc")
                n4 = min(4, nt - t4)
                for j in range(n4):
                    t = t4 + j
                    nc.tensor.transpose(
                        ptr[:, 32 * j:32 * j + 32],
                        uT_aug[0:BITS, t * P:(t + 1) * P],
                        ident_sb[0:BITS, 0:BITS],
                    )
                for j in range(n4):
                    t = t4 + j
                    nc.vector.tensor_copy(
                        u_norm[:, t * 33:t * 33 + 32],
                        ptr[:, 32 * j:32 * j + 32],
                    )

            # T totals: accumulate u_aug^T @ ones over all tiles; AllReduce
            ones_col = constp.tile([P, 1], bf16)
            ones_1x32 = constp.tile([1, BITS], bf16)
            nc.vector.memset(ones_col[:], 1.0)
            nc.vector.memset(ones_1x32[:], 1.0)
            pT = psS.tile([33, 1], f32, tag="misc")
            for t in range(nt):
                nc.tensor.matmul(pT[:], u_norm[:, t * 33:(t + 1) * 33],
                                 ones_col[:], start=(t == 0), stop=(t == nt - 1))
            T_loc = workp.tile([33, 1], f32, tag="Tloc")
            nc.vector.tensor_copy(T_loc[:], pT[:])
            tcc_in = dramp.tile([33, 1], f32, tag="tccin")
            tcc_out = dramp.tile([33, 1], f32, tag="tccout")
            nc.sync.dma_start(tcc_in[:], T_loc[:])
            nc.gpsimd.collective_compute(
                "AllReduce", OP.add, replica_groups=rg,
                ins=[tcc_in.opt()], outs=[tcc_out.opt()],
            )
            nc.sync.dma_start(T_sb[:], tcc_out[:])

            # ---------------- iterations ----------------
            def assign_pass(W_a, W_b, save_mx=False):
                """Row-tiled assign + stats pass. Returns stats psum [33, K]."""
                ps_stats = psS.tile([33, K], f32, tag="stats")
                for t in range(nt):
                    pa = psA.tile([P, K], f32, tag="assign")
                    if t % 2 == 0:
                        nc.tensor.matmul(
                            pa[:], uT_aug[:, t * P:(t + 1) * P],
                            W_a[:], start=True, stop=True,
                            tile_position=(0, 0),
                        )
                    else:
                        nc.tensor.matmul(
                            pa[:], uT_aug2[64:64 + CR, t * P:(t + 1) * P],
                            W_b[64:64 + CR, :], start=True, stop=True,
                            tile_position=(64, 0),
                        )
                    mx = hp.tile([P, 1], f32, tag="mx")
                    nc.vector.tensor_reduce(mx[:], pa[:], axis=AX.X, op=OP.max)
                    if save_mx:
                        nc.vector.tensor_copy(mxall[:, t:t + 1], mx[:])
                    h = hp.tile([P, K], bf16, tag="h")
                    nc.scalar.activation(
                        h[:], pa[:], ACT.Sign, bias=mx[:], scale=-1.0,
                    )
                    nc.tensor.matmul(
                        ps_stats[:], u_norm[:, t * 33:(t + 1) * 33], h[:],
                        start=(t == 0), stop=(t == nt - 1),
                    )
                return ps_stats

            def reduce_stats(ps_stats):
                """AllGather stats across cores + local sum -> S sbuf [33,K]."""
                cc_in = dramp.tile([33, K], f32, tag="ccin")
                cc_out = dramp.tile([N_CORES * 33, K], f32, tag="ccout")
                S_loc = workp.tile([33, K], f32, tag="Sloc")
                nc.scalar.activation(S_loc[:], ps_stats[:], ACT.Copy)
                nc.sync.dma_start(cc_in[:], S_loc[:])
                nc.gpsimd.collective_compute(
                    "AllGather", OP.bypass, replica_groups=rg,
                    ins=[cc_in.opt()], outs=[cc_out.opt()],
                )
                g8 = workp.tile([33, N_CORES * K], f32, tag="g8")
                nc.sync.dma_start(
                    g8[:].rearrange("j (r m) -> j r m", m=K),
                    cc_out[:].rearrange("(r j) m -> j r m", j=33),
                )
                # tree-sum the 8 rank blocks
                nc.vector.tensor_tensor(g8[:, 0:4 * K], g8[:, 0:4 * K],
                                        g8[:, 4 * K:8 * K], op=OP.add)
                nc.vector.tensor_tensor(g8[:, 0:2 * K], g8[:, 0:2 * K],
                                        g8[:, 2 * K:4 * K], op=OP.add)
                S = workp.tile([33, K], f32, tag="S")
                nc.vector.tensor_tensor(S[:], g8[:, 0:K], g8[:, K:2 * K],
                                        op=OP.add)
                return S

            def make_W64(W_t):
                W64 = workp.tile([64 + CR, K], bf16, tag="W64")
                nc.sync.dma_start(W64[64:64 + CR, :], W_t[:])
                return W64

            for it in range(N_ITER):
                W64 = make_W64(W_cur)
                ps_stats = assign_pass(W_cur, W64)
                S = reduce_stats(ps_stats)

                # V = S - T = -true_stats
                V = workp.tile([33, K], f32, tag="V")
                nc.vector.tensor_scalar(V[:], S[:], T_sb[:], None,
                                        op0=OP.subtract)
                t01 = workp.tile([BITS, K], bf16, tag="t01")
                nc.vector.tensor_scalar(t01[:], V[0:BITS, :], 0.0, None,
                                        op0=OP.is_lt)
                # W_new rows = t01*2048 - 1024 in {-1024, +1024}
                W_next = workp.tile([CR, K], bf16, tag="W")
                nc.vector.tensor_scalar(W_next[0:BITS, :], t01[:], 2048.0,
                                        -1024.0, op0=OP.mult, op1=OP.add)
                nc.sync.dma_start(W_next[BITS:CR, :], wrows[:])
                # empty-cluster mask (count == 0 <=> V[32] == 0)
                cm = workp.tile([1, K], bf16, tag="cm")
                nc.vector.tensor_scalar(cm[:], V[32:33, :], 0.0, None,
                                        op0=OP.is_equal)
                pb = psS.tile([BITS, K], f32, tag="misc")
                nc.tensor.matmul(pb[:], ones_1x32[:], cm[:],
                                 start=True, stop=True)
                mask_u8 = workp.tile([BITS, K], u8, tag="mask")
                nc.vector.tensor_copy(mask_u8[:], pb[:])
                nc.vector.copy_predicated(W_next[0:BITS, :], mask_u8[:],
                                          W_cur[0:BITS, :])
                W_cur = W_next

            # ---------------- final assign + outputs ----------------
            W64 = make_W64(W_cur)
            ps_stats = assign_pass(W_cur, W64, save_mx=True)
            S = reduce_stats(ps_stats)

            # counts = -(S - T)[32]
            V = workp.tile([33, K], f32, tag="V")
            nc.vector.tensor_scalar(V[:], S[:], T_sb[:], None, op0=OP.subtract)
            cnt_f = workp.tile([1, K], f32, tag="cntf")
            nc.vector.tensor_scalar(cnt_f[:], V[32:33, :], -1.0, None,
                                    op0=OP.mult)
            cnt_i = workp.tile([1, K], i32, tag="cnti")
            nc.vector.tensor_copy(cnt_i[:], cnt_f[:])
            nc.sync.dma_start(counts_d[:], cnt_i[:])

            # labels: a = 511 - (mx & 1023) via integer ops (mod unsupported)
            lab_i32 = bigp.tile([P, nt], i32)
            nc.vector.tensor_copy(lab_i32[:], mxall[:])
            nc.vector.tensor_scalar(lab_i32[:], lab_i32[:], 1023, None,
                                    op0=OP.bitwise_and)
            lab_f = bigp.tile([P, nt], f32)
            nc.vector.tensor_copy(lab_f[:], lab_i32[:])
            nc.vector.tensor_scalar(lab_f[:], lab_f[:], -1.0, 511.0,
                                    op0=OP.mult, op1=OP.add)
            for t4 in range(0, nt, P):   # transpose nt columns in P-blocks
                w = min(P, nt - t4)
                ptr = psS.tile([P, P], f32, tag="misc")
                nc.tensor.transpose(ptr[0:w, 0:P], lab_f[:, t4:t4 + w],
                                    identf_sb[:])
                lab_i = workp.tile([P, P], i32, tag="labi")
                nc.vector.tensor_copy(lab_i[0:w, :], ptr[0:w, 0:P])
                nc.sync.dma_start(labels_d[t4:t4 + w, :], lab_i[0:w, :])

    nc.finalize()
    return nc


def _prep_inputs(x, planes):
    """Host-side sharding/layout prep (numpy only)."""
    import concourse.mybir as mybir
    bf = mybir.dt.np(mybir.dt.bfloat16)
    xT = np.ascontiguousarray(x.T, dtype=np.float32)          # [E, L]
    xinit_t = np.ascontiguousarray(x[::L // K].T, dtype=np.float32)  # [E, K]
    planes_t = np.ascontiguousarray(planes.T, dtype=np.float32)      # [E+1, 32]
    wrows = np.stack([_WROW_HI, _WROW_LO]).astype(bf)         # [2, K]
    ident_b = np.eye(P, dtype=np.float32).astype(bf)
    ident_f = np.eye(P, dtype=np.float32)
    in_maps = []
    for c in range(N_CORES):
        in_maps.append({
            "xt": np.ascontiguousarray(xT[:, c * LP:(c + 1) * LP]),
            "xinit_t": xinit_t,
            "planes_t": planes_t,
            "wrows": wrows,
            "ident_b": ident_b,
            "ident_f": ident_f,
        })
    return in_maps


_NC_CACHE = {}

LAST_EXEC_NS = None


def kernel(x, planes, k):
    import os
    from concourse.bass_utils import run_bass_kernel_spmd

    global LAST_EXEC_NS
    assert int(k) == K
    x = np.asarray(x, dtype=np.float32)
    planes = np.asarray(planes, dtype=np.float32)
    in_maps = _prep_inputs(x, planes)

    nt = LP // P
    if nt not in _NC_CACHE:
        _NC_CACHE[nt] = build_kernel(nt)
    nc = _NC_CACHE[nt]

    trace = bool(int(os.environ.get("BASS_KERNEL_TRACE", "0")))
    res = run_bass_kernel_spmd(nc, in_maps, core_ids=list(range(N_CORES)),
                               trace=trace)
    LAST_EXEC_NS = res.exec_time_ns
    outs = res.results
    labels = np.concatenate(
        [outs[c]["labels"].reshape(-1) for c in range(N_CORES)]
    ).astype(np.int32)
    counts = outs[0]["counts"].reshape(-1).astype(np.int32)
    return labels, counts


# revision 22
# speedup vs baseline: 1.2974x; 1.2974x over previous
"""Distributed Hamming k-means (LSH moe_routing) Bass kernel for 8 TRN2 cores.

Algorithm notes (exactness):
- bits b = (x@P.T + bias > 0); work in u = 2b-1 (+-1) space.
- Hamming dist d(p,k) = (32 - u_p . w_k)/2, so argmin_k d = argmax_k dot.
- Assign matmul computes combined = 1024*dot + 32768 + (511-k) using two
  extra all-ones contraction rows (W rows store +-1024 and the encode rows);
  every term is an exactly-representable integer, so argmax(combined)
  reproduces argmin-with-first-index-tie-break exactly.
- Inverted one-hot h = Sign(max - combined) in {0(argmax), +1}; stats matmul
  S_h = u_aug^T @ h, true stats = T - sum_cores(S_h) with T = u_aug col sums
  (T is AllReduced once; per-iter stats use AllGather + on-device sum).
- Vote: new w = +1024 iff votesum > 0; empty clusters keep old w.

Perf structure:
- Assign matmuls are 2-way row-tiled (contraction 34 <= 64): tile A uses PE
  rows 0-63 with lhsT/rhs at partition 0, tile B uses rows 64-127 with
  SBUF copies of uT_aug/W at partition 64 (replicated via SBUF->SBUF DMA).
- PSUM: 3x [128,1024] assign pairs + 2x shared misc/stats bank = 8 banks.
"""

import numpy as np

L, E, BITS, K = 131072, 256, 32, 512
N_CORES, N_ITER = 8, 10
LP = L // N_CORES            # 16384 points per core
P = 128                      # partitions / tile height
PAIR_F = 1024                # assign psum free size (2 banks, 2 tiles)
CR = BITS + 2                # assign contraction rows (32 bits + 2 encode)

_ENC = 511 - np.arange(K)    # tie-break code per cluster
_WROW_HI = 32768.0 + 256.0 * (_ENC >> 8)   # bf16-exact
_WROW_LO = (_ENC & 255).astype(np.float64)  # bf16-exact


def build_kernel(nt):
    """Build the SPMD Bass program. nt = number of 128-point tiles per core."""
    import concourse.bacc as bacc
    import concourse.mybir as mybir
    import concourse.tile as tile

    lp = nt * P
    f32 = mybir.dt.float32
    bf16 = mybir.dt.bfloat16
    i32 = mybir.dt.int32
    u8 = mybir.dt.uint8
    AX = mybir.AxisListType
    OP = mybir.AluOpType
    ACT = mybir.ActivationFunctionType

    nc = bacc.Bacc(None, target_bir_lowering=False, debug=False)

    xt = nc.dram_tensor("xt", [E, lp], f32, kind="ExternalInput")
    xinit_t = nc.dram_tensor("xinit_t", [E, K], f32, kind="ExternalInput")
    planes_t = nc.dram_tensor("planes_t", [E + 1, BITS], f32, kind="ExternalInput")
    wrows = nc.dram_tensor("wrows", [2, K], bf16, kind="ExternalInput")
    ident_b = nc.dram_tensor("ident_b", [P, P], bf16, kind="ExternalInput")
    ident_f = nc.dram_tensor("ident_f", [P, P], f32, kind="ExternalInput")
    labels_d = nc.dram_tensor("labels", [nt, P], i32, kind="ExternalOutput")
    counts_d = nc.dram_tensor("counts", [1, K], i32, kind="ExternalOutput")

    rg = [list(range(N_CORES))]

    with tile.TileContext(nc) as tc:
        with (
            tc.tile_pool(name="const", bufs=1) as constp,
            tc.tile_pool(name="big", bufs=1) as bigp,
            tc.tile_pool(name="work", bufs=3) as workp,
            tc.tile_pool(name="htile", bufs=4) as hp,
            tc.tile_pool(name="psA", bufs=3, space="PSUM") as psA,
            tc.tile_pool(name="psS", bufs=1, space="PSUM") as psS,
            tc.tile_pool(name="dram", bufs=2, space="DRAM") as dramp,
        ):
            # ---------------- static tiles ----------------
            # uT_aug rows 0-31: u codes (transposed); rows 32,33: ones
            uT_aug = bigp.tile([CR, lp], bf16)
            # replica at partition 64 for row-tiled matmul B
            uT_aug2 = bigp.tile([64 + CR, lp], bf16)
            # u_norm: per tile [128, 33] = [u | 1] blocks, 33-strided
            u_norm = bigp.tile([P, nt * 33], bf16)
            ident_sb = constp.tile([P, P], bf16)
            identf_sb = constp.tile([P, P], f32)
            pt0 = constp.tile([P, BITS], f32)           # planes_t rows 0:128
            pt1 = constp.tile([P, BITS], f32)           # planes_t rows 128:256
            ptb = constp.tile([1, BITS], f32)           # bias row
            ones_row = constp.tile([1, K], f32)         # bias-row rhs
            T_sb = constp.tile([33, 1], f32)            # global u_aug col totals
            mxall = bigp.tile([P, nt], f32)             # per-tile max values

            nc.sync.dma_start(ident_sb[:], ident_b[:])
            nc.sync.dma_start(identf_sb[:], ident_f[:])
            nc.sync.dma_start(pt0[:], planes_t[0:P, :])
            nc.sync.dma_start(pt1[:], planes_t[P:2 * P, :])
            nc.sync.dma_start(ptb[:], planes_t[2 * P:2 * P + 1, :])
            nc.vector.memset(ones_row[:], 1.0)
            nc.vector.memset(uT_aug[BITS:CR, :], 1.0)
            nc.vector.memset(u_norm[:], 1.0)

            # ---------------- phase H: hash ----------------
            def hash_chunk(dst_ap, src_cols_a, src_cols_b, width):
                """proj^T for `width` points -> Sign -> dst_ap (bf16 +-1)."""
                pp = psS.tile([BITS, width], f32, tag="misc")
                nc.tensor.matmul(pp[:], pt0[:], src_cols_a, start=True,
                                 stop=False)
                nc.tensor.matmul(pp[:], pt1[:], src_cols_b,
                                 start=False, stop=False)
                nc.tensor.matmul(pp[:], ptb[:],
                                 ones_row[:, 0:width], start=False, stop=True)
                nc.scalar.activation(dst_ap, pp[:], ACT.Sign)

            CH = 512
            for j in range(0, lp, CH):
                xa = workp.tile([P, CH], f32, tag="xa")
                xb = workp.tile([P, CH], f32, tag="xb")
                nc.sync.dma_start(xa[:], xt[0:P, j:j + CH])
                nc.sync.dma_start(xb[:], xt[P:2 * P, j:j + CH])
                hash_chunk(uT_aug[0:BITS, j:j + CH], xa[:], xb[:], CH)

            # replicate codes+ones to partition 64 (SBUF->SBUF DMA)
            nc.sync.dma_start(uT_aug2[64:64 + CR, :], uT_aug[:])

            # C0 from the 512 init points (replicated input); stored +-1024
            W_sgn = workp.tile([BITS, K], bf16, tag="Wsgn")
            xa = workp.tile([P, K], f32, tag="xa")
            xb = workp.tile([P, K], f32, tag="xb")
            nc.sync.dma_start(xa[:], xinit_t[0:P, :])
            nc.sync.dma_start(xb[:], xinit_t[P:2 * P, :])
            hash_chunk(W_sgn[:], xa[:], xb[:], K)
            W_cur = workp.tile([CR, K], bf16, tag="W")
            nc.vector.tensor_scalar(W_cur[0:BITS, :], W_sgn[:], 1024.0, None,
                                    op0=OP.mult)
            nc.sync.dma_start(W_cur[BITS:CR, :], wrows[:])

            # u_norm via PE transposes (4 tiles share one psum buffer)
            for t4 in range(0, nt, 4):
                ptr = psS.tile([P, P], bf16, tag="misc")
                n4 = min(4, nt - t4)
                for j in range(n4):
                    t = t4 + j
                    nc.tensor.transpose(
                        ptr[:, 32 * j:32 * j + 32],
                        uT_aug[0:BITS, t * P:(t + 1) * P],
                        ident_sb[0:BITS, 0:BITS],
                    )
                for j in range(n4):
                    t = t4 + j
                    nc.vector.tensor_copy(
                        u_norm[:, t * 33:t * 33 + 32],
                        ptr[:, 32 * j:32 * j + 32],
                    )

            # T totals: accumulate u_aug^T @ ones over all tiles; AllReduce
            ones_col = constp.tile([P, 1], bf16)
            ones_1x32 = constp.tile([1, BITS], bf16)
            nc.vector.memset(ones_col[:], 1.0)
            nc.vector.memset(ones_1x32[:], 1.0)
            pT = psS.tile([33, 1], f32, tag="misc")
            for t in range(nt):
                nc.tensor.matmul(pT[:], u_norm[:, t * 33:(t + 1) * 33],
                                 ones_col[:], start=(t == 0), stop=(t == nt - 1))
            T_loc = workp.tile([33, 1], f32, tag="Tloc")
            nc.vector.tensor_copy(T_loc[:], pT[:])
            tcc_in = dramp.tile([33, 1], f32, tag="tccin")
            tcc_out = dramp.tile([33, 1], f32, tag="tccout")
            nc.sync.dma_start(tcc_in[:], T_loc[:])
            nc.gpsimd.collective_compute(
                "AllReduce", OP.add, replica_groups=rg,
                ins=[tcc_in.opt()], outs=[tcc_out.opt()],
            )
            nc.sync.dma_start(T_sb[:], tcc_out[:])

            # ---------------- iterations ----------------
            def assign_pass(W_a, W_b, save_mx=False):
                """Row-tiled assign + stats pass. Returns stats psum [33, K]."""
                ps_stats = psS.tile([33, K], f32, tag="stats")
                for tp in range(0, nt, 2):   # pairs of tiles -> 2 psum banks
                    pa = psA.tile([P, PAIR_F], f32, tag="assign")
                    npair = min(2, nt - tp)
                    for j in range(npair):
                        t = tp + j
                        if j == 0:
                            nc.tensor.matmul(
                                pa[:, 0:K],
                                uT_aug[:, t * P:(t + 1) * P],
                                W_a[:], start=True, stop=True,
                                tile_position=(0, 0),
                            )
                        else:
                            nc.tensor.matmul(
                                pa[:, K:2 * K],
                                uT_aug2[64:64 + CR, t * P:(t + 1) * P],
                                W_b[64:64 + CR, :], start=True, stop=True,
                                tile_position=(64, 0),
                            )
                    nc.vector.tensor_reduce(
                        mxall[:, tp:tp + npair],
                        pa[:].rearrange("p (n f) -> p n f", f=K)[:, 0:npair, :],
                        axis=AX.X, op=OP.max,
                    )
                    for j in range(npair):
                        t = tp + j
                        h = hp.tile([P, K], bf16, tag="h")
                        nc.scalar.activation(
                            h[:], pa[:, j * K:(j + 1) * K], ACT.Sign,
                            bias=mxall[:, t:t + 1], scale=-1.0,
                        )
                        nc.tensor.matmul(
                            ps_stats[:], u_norm[:, t * 33:(t + 1) * 33], h[:],
                            start=(t == 0), stop=(t == nt - 1),
                        )
                return ps_stats

            def reduce_stats(ps_stats):
                """AllGather stats across cores + local sum -> S sbuf [33,K]."""
                cc_in = dramp.tile([33, K], f32, tag="ccin")
                cc_out = dramp.tile([N_CORES * 33, K], f32, tag="ccout")
                S_loc = workp.tile([33, K], f32, tag="Sloc")
                nc.scalar.activation(S_loc[:], ps_stats[:], ACT.Copy)
                nc.sync.dma_start(cc_in[:], S_loc[:])
                nc.gpsimd.collective_compute(
                    "AllGather", OP.bypass, replica_groups=rg,
                    ins=[cc_in.opt()], outs=[cc_out.opt()],
                )
                g8 = workp.tile([33, N_CORES * K], f32, tag="g8")
                nc.sync.dma_start(
                    g8[:].rearrange("j (r m) -> j r m", m=K),
                    cc_out[:].rearrange("(r j) m -> j r m", j=33),
                )
                # tree-sum the 8 rank blocks
                nc.vector.tensor_tensor(g8[:, 0:4 * K], g8[:, 0:4 * K],
                                        g8[:, 4 * K:8 * K], op=OP.add)
                nc.vector.tensor_tensor(g8[:, 0:2 * K], g8[:, 0:2 * K],
                                        g8[:, 2 * K:4 * K], op=OP.add)
                S = workp.tile([33, K], f32, tag="S")
                nc.vector.tensor_tensor(S[:], g8[:, 0:K], g8[:, K:2 * K],
                                        op=OP.add)
                return S

            def make_W64(W_t):
                W64 = workp.tile([64 + CR, K], bf16, tag="W64")
                nc.sync.dma_start(W64[64:64 + CR, :], W_t[:])
                return W64

            for it in range(N_ITER):
                W64 = make_W64(W_cur)
                ps_stats = assign_pass(W_cur, W64)
                S = reduce_stats(ps_stats)

                # V = S - T = -true_stats
                V = workp.tile([33, K], f32, tag="V")
                nc.vector.tensor_scalar(V[:], S[:], T_sb[:], None,
                                        op0=OP.subtract)
                t01 = workp.tile([BITS, K], bf16, tag="t01")
                nc.vector.tensor_scalar(t01[:], V[0:BITS, :], 0.0, None,
                                        op0=OP.is_lt)
                # W_new rows = t01*2048 - 1024 in {-1024, +1024}
                W_next = workp.tile([CR, K], bf16, tag="W")
                nc.vector.tensor_scalar(W_next[0:BITS, :], t01[:], 2048.0,
                                        -1024.0, op0=OP.mult, op1=OP.add)
                nc.sync.dma_start(W_next[BITS:CR, :], wrows[:])
                # empty-cluster mask (count == 0 <=> V[32] == 0)
                cm = workp.tile([1, K], bf16, tag="cm")
                nc.vector.tensor_scalar(cm[:], V[32:33, :], 0.0, None,
                                        op0=OP.is_equal)
                pb = psS.tile([BITS, K], f32, tag="misc")
                nc.tensor.matmul(pb[:], ones_1x32[:], cm[:],
                                 start=True, stop=True)
                mask_u8 = workp.tile([BITS, K], u8, tag="mask")
                nc.vector.tensor_copy(mask_u8[:], pb[:])
                nc.vector.copy_predicated(W_next[0:BITS, :], mask_u8[:],
                                          W_cur[0:BITS, :])
                W_cur = W_next

            # ---------------- final assign + outputs ----------------
            W64 = make_W64(W_cur)
            ps_stats = assign_pass(W_cur, W64, save_mx=True)
            S = reduce_stats(ps_stats)

            # counts = -(S - T)[32]
            V = workp.tile([33, K], f32, tag="V")
            nc.vector.tensor_scalar(V[:], S[:], T_sb[:], None, op0=OP.subtract)
            cnt_f = workp.tile([1, K], f32, tag="cntf")
            nc.vector.tensor_scalar(cnt_f[:], V[32:33, :], -1.0, None,
                                    op0=OP.mult)
            cnt_i = workp.tile([1, K], i32, tag="cnti")
            nc.vector.tensor_copy(cnt_i[:], cnt_f[:])
            nc.sync.dma_start(counts_d[:], cnt_i[:])

            # labels: a = 511 - (mx & 1023) via integer ops (mod unsupported)
            lab_i32 = bigp.tile([P, nt], i32)
            nc.vector.tensor_copy(lab_i32[:], mxall[:])
            nc.vector.tensor_scalar(lab_i32[:], lab_i32[:], 1023, None,
                                    op0=OP.bitwise_and)
            lab_f = bigp.tile([P, nt], f32)
            nc.vector.tensor_copy(lab_f[:], lab_i32[:])
            nc.vector.tensor_scalar(lab_f[:], lab_f[:], -1.0, 511.0,
                                    op0=OP.mult, op1=OP.add)
            for t4 in range(0, nt, P):   # transpose nt columns in P-blocks
                w = min(P, nt - t4)
                ptr = psS.tile([P, P], f32, tag="misc")
                nc.tensor.transpose(ptr[0:w, 0:P], lab_f[:, t4:t4 + w],
                                    identf_sb[:])
                lab_i = workp.tile([P, P], i32, tag="labi")
                nc.vector.tensor_copy(lab_i[0:w, :], ptr[0:w, 0:P])
                nc.sync.dma_start(labels_d[t4:t4 + w, :], lab_i[0:w, :])

    nc.finalize()
    return nc


def _prep_inputs(x, planes):
    """Host-side sharding/layout prep (numpy only)."""
    import concourse.mybir as mybir
    bf = mybir.dt.np(mybir.dt.bfloat16)
    xT = np.ascontiguousarray(x.T, dtype=np.float32)          # [E, L]
    xinit_t = np.ascontiguousarray(x[::L // K].T, dtype=np.float32)  # [E, K]
    planes_t = np.ascontiguousarray(planes.T, dtype=np.float32)      # [E+1, 32]
    wrows = np.stack([_WROW_HI, _WROW_LO]).astype(bf)         # [2, K]
    ident_b = np.eye(P, dtype=np.float32).astype(bf)
    ident_f = np.eye(P, dtype=np.float32)
    in_maps = []
    for c in range(N_CORES):
        in_maps.append({
            "xt": np.ascontiguousarray(xT[:, c * LP:(c + 1) * LP]),
            "xinit_t": xinit_t,
            "planes_t": planes_t,
            "wrows": wrows,
            "ident_b": ident_b,
            "ident_f": ident_f,
        })
    return in_maps


_NC_CACHE = {}

LAST_EXEC_NS = None


def kernel(x, planes, k):
    import os
    from concourse.bass_utils import run_bass_kernel_spmd

    global LAST_EXEC_NS
    assert int(k) == K
    x = np.asarray(x, dtype=np.float32)
    planes = np.asarray(planes, dtype=np.float32)
    in_maps = _prep_inputs(x, planes)

    nt = LP // P
    if nt not in _NC_CACHE:
        _NC_CACHE[nt] = build_kernel(nt)
    nc = _NC_CACHE[nt]

    trace = bool(int(os.environ.get("BASS_KERNEL_TRACE", "0")))
    res = run_bass_kernel_spmd(nc, in_maps, core_ids=list(range(N_CORES)),
                               trace=trace)
    LAST_EXEC_NS = res.exec_time_ns
    outs = res.results
    labels = np.concatenate(
        [outs[c]["labels"].reshape(-1) for c in range(N_CORES)]
    ).astype(np.int32)
    counts = outs[0]["counts"].reshape(-1).astype(np.int32)
    return labels, counts


# revision 23
# speedup vs baseline: 1.3065x; 1.0070x over previous
"""Distributed Hamming k-means (LSH moe_routing) Bass kernel for 8 TRN2 cores.

Algorithm notes (exactness):
- bits b = (x@P.T + bias > 0); work in u = 2b-1 (+-1) space.
- Hamming dist d(p,k) = (32 - u_p . w_k)/2, so argmin_k d = argmax_k dot.
- Assign matmul computes combined = 1024*dot + 32768 + (511-k) using two
  extra all-ones contraction rows (W rows store +-1024 and the encode rows);
  every term is an exactly-representable integer, so argmax(combined)
  reproduces argmin-with-first-index-tie-break exactly.
- Inverted one-hot h = Sign(max - combined) in {0(argmax), +1}; stats matmul
  S_h = u_aug^T @ h, true stats = T - sum_cores(S_h) with T = u_aug col sums
  (T is AllReduced once; per-iter stats use AllGather + on-device sum).
- Vote: new w = +1024 iff votesum > 0; empty clusters keep old w.

Perf structure:
- Assign matmuls are 2-way row-tiled (contraction 34 <= 64): tile A uses PE
  rows 0-63 with lhsT/rhs at partition 0, tile B uses rows 64-127 with
  SBUF copies of uT_aug/W at partition 64 (replicated via SBUF->SBUF DMA).
- PSUM: 3x [128,1024] assign pairs + 2x shared misc/stats bank = 8 banks.
"""

import numpy as np

L, E, BITS, K = 131072, 256, 32, 512
N_CORES, N_ITER = 8, 10
LP = L // N_CORES            # 16384 points per core
P = 128                      # partitions / tile height
PAIR_F = 1024                # assign psum free size (2 banks, 2 tiles)
CR = BITS + 2                # assign contraction rows (32 bits + 2 encode)

_ENC = 511 - np.arange(K)    # tie-break code per cluster
_WROW_HI = 32768.0 + 256.0 * (_ENC >> 8)   # bf16-exact
_WROW_LO = (_ENC & 255).astype(np.float64)  # bf16-exact


def build_kernel(nt):
    """Build the SPMD Bass program. nt = number of 128-point tiles per core."""
    import concourse.bacc as bacc
    import concourse.mybir as mybir
    import concourse.tile as tile

    lp = nt * P
    f32 = mybir.dt.float32
    bf16 = mybir.dt.bfloat16
    i32 = mybir.dt.int32
    u8 = mybir.dt.uint8
    AX = mybir.AxisListType
    OP = mybir.AluOpType
    ACT = mybir.ActivationFunctionType

    nc = bacc.Bacc(None, target_bir_lowering=False, debug=False)

    xt = nc.dram_tensor("xt", [E, lp], f32, kind="ExternalInput")
    xinit_t = nc.dram_tensor("xinit_t", [E, K], f32, kind="ExternalInput")
    planes_t = nc.dram_tensor("planes_t", [E + 1, BITS], f32, kind="ExternalInput")
    wrows = nc.dram_tensor("wrows", [2, K], bf16, kind="ExternalInput")
    ident_b = nc.dram_tensor("ident_b", [P, P], bf16, kind="ExternalInput")
    ident_f = nc.dram_tensor("ident_f", [P, P], f32, kind="ExternalInput")
    labels_d = nc.dram_tensor("labels", [nt, P], i32, kind="ExternalOutput")
    counts_d = nc.dram_tensor("counts", [1, K], i32, kind="ExternalOutput")

    rg = [list(range(N_CORES))]

    with tile.TileContext(nc) as tc:
        with (
            tc.tile_pool(name="const", bufs=1) as constp,
            tc.tile_pool(name="big", bufs=1) as bigp,
            tc.tile_pool(name="work", bufs=3) as workp,
            tc.tile_pool(name="htile", bufs=4) as hp,
            tc.tile_pool(name="psA", bufs=3, space="PSUM") as psA,
            tc.tile_pool(name="psS", bufs=1, space="PSUM") as psS,
            tc.tile_pool(name="dram", bufs=2, space="DRAM") as dramp,
        ):
            # ---------------- static tiles ----------------
            # uT_aug rows 0-31: u codes (transposed); rows 32,33: ones
            uT_aug = bigp.tile([CR, lp], bf16)
            # replica at partition 64 for row-tiled matmul B
            uT_aug2 = bigp.tile([64 + CR, lp], bf16)
            # u_norm: per tile [128, 33] = [u | 1] blocks, 33-strided
            u_norm = bigp.tile([P, nt * 33], bf16)
            ident_sb = constp.tile([P, P], bf16)
            identf_sb = constp.tile([P, P], f32)
            pt0 = constp.tile([P, BITS], f32)           # planes_t rows 0:128
            pt1 = constp.tile([P, BITS], f32)           # planes_t rows 128:256
            ptb = constp.tile([1, BITS], f32)           # bias row
            ones_row = constp.tile([1, K], f32)         # bias-row rhs
            T_sb = constp.tile([33, 1], f32)            # global u_aug col totals
            mxall = bigp.tile([P, nt], f32)             # per-tile max values

            nc.sync.dma_start(ident_sb[:], ident_b[:])
            nc.sync.dma_start(identf_sb[:], ident_f[:])
            nc.sync.dma_start(pt0[:], planes_t[0:P, :])
            nc.sync.dma_start(pt1[:], planes_t[P:2 * P, :])
            nc.sync.dma_start(ptb[:], planes_t[2 * P:2 * P + 1, :])
            nc.vector.memset(ones_row[:], 1.0)
            nc.vector.memset(uT_aug[BITS:CR, :], 1.0)
            nc.vector.memset(u_norm[:], 1.0)

            # ---------------- phase H: hash ----------------
            def hash_chunk(dst_ap, src_cols_a, src_cols_b, width):
                """proj^T for `width` points -> Sign -> dst_ap (bf16 +-1)."""
                pp = psS.tile([BITS, width], f32, tag="misc")
                nc.tensor.matmul(pp[:], pt0[:], src_cols_a, start=True,
                                 stop=False)
                nc.tensor.matmul(pp[:], pt1[:], src_cols_b,
                                 start=False, stop=False)
                nc.tensor.matmul(pp[:], ptb[:],
                                 ones_row[:, 0:width], start=False, stop=True)
                nc.scalar.activation(dst_ap, pp[:], ACT.Sign)

            CH = 512
            for j in range(0, lp, CH):
                xa = workp.tile([P, CH], f32, tag="xa")
                xb = workp.tile([P, CH], f32, tag="xb")
                nc.sync.dma_start(xa[:], xt[0:P, j:j + CH])
                nc.sync.dma_start(xb[:], xt[P:2 * P, j:j + CH])
                hash_chunk(uT_aug[0:BITS, j:j + CH], xa[:], xb[:], CH)

            # replicate codes+ones to partition 64 (SBUF->SBUF DMA)
            nc.sync.dma_start(uT_aug2[64:64 + CR, :], uT_aug[:])

            # C0 from the 512 init points (replicated input); stored +-1024
            W_sgn = workp.tile([BITS, K], bf16, tag="Wsgn")
            xa = workp.tile([P, K], f32, tag="xa")
            xb = workp.tile([P, K], f32, tag="xb")
            nc.sync.dma_start(xa[:], xinit_t[0:P, :])
            nc.sync.dma_start(xb[:], xinit_t[P:2 * P, :])
            hash_chunk(W_sgn[:], xa[:], xb[:], K)
            W_cur = workp.tile([CR, K], bf16, tag="W")
            nc.vector.tensor_scalar(W_cur[0:BITS, :], W_sgn[:], 1024.0, None,
                                    op0=OP.mult)
            nc.sync.dma_start(W_cur[BITS:CR, :], wrows[:])

            # u_norm via PE transposes (4 tiles share one psum buffer)
            for t4 in range(0, nt, 4):
                ptr = psS.tile([P, P], bf16, tag="misc")
                n4 = min(4, nt - t4)
                for j in range(n4):
                    t = t4 + j
                    nc.tensor.transpose(
                        ptr[:, 32 * j:32 * j + 32],
                        uT_aug[0:BITS, t * P:(t + 1) * P],
                        ident_sb[0:BITS, 0:BITS],
                    )
                for j in range(n4):
                    t = t4 + j
                    nc.vector.tensor_copy(
                        u_norm[:, t * 33:t * 33 + 32],
                        ptr[:, 32 * j:32 * j + 32],
                    )

            # T totals: accumulate u_aug^T @ ones over all tiles; AllReduce
            ones_col = constp.tile([P, 1], bf16)
            ones_1x32 = constp.tile([1, BITS], bf16)
            nc.vector.memset(ones_col[:], 1.0)
            nc.vector.memset(ones_1x32[:], 1.0)
            pT = psS.tile([33, 1], f32, tag="misc")
            for t in range(nt):
                nc.tensor.matmul(pT[:], u_norm[:, t * 33:(t + 1) * 33],
                                 ones_col[:], start=(t == 0), stop=(t == nt - 1))
            T_loc = workp.tile([33, 1], f32, tag="Tloc")
            nc.vector.tensor_copy(T_loc[:], pT[:])
            tcc_in = dramp.tile([33, 1], f32, tag="tccin")
            tcc_out = dramp.tile([33, 1], f32, tag="tccout")
            nc.sync.dma_start(tcc_in[:], T_loc[:])
            nc.gpsimd.collective_compute(
                "AllReduce", OP.add, replica_groups=rg,
                ins=[tcc_in.opt()], outs=[tcc_out.opt()],
            )
            nc.sync.dma_start(T_sb[:], tcc_out[:])

            # ---------------- iterations ----------------
            def assign_pass(W_a, W_b, save_mx=False):
                """Row-tiled assign + stats pass. Returns stats psum [33, K]."""
                ps_stats = psS.tile([33, K], f32, tag="stats")
                for tp in range(0, nt, 2):   # pairs of tiles -> 2 psum banks
                    pa = psA.tile([P, PAIR_F], f32, tag="assign")
                    npair = min(2, nt - tp)
                    for j in range(npair):
                        t = tp + j
                        if j == 0:
                            nc.tensor.matmul(
                                pa[:, 0:K],
                                uT_aug[:, t * P:(t + 1) * P],
                                W_a[:], start=True, stop=True,
                                tile_position=(0, 0),
                            )
                        else:
                            nc.tensor.matmul(
                                pa[:, K:2 * K],
                                uT_aug2[64:64 + CR, t * P:(t + 1) * P],
                                W_b[64:64 + CR, :], start=True, stop=True,
                                tile_position=(64, 0),
                            )
                    nc.vector.tensor_reduce(
                        mxall[:, tp:tp + npair],
                        pa[:].rearrange("p (n f) -> p n f", f=K)[:, 0:npair, :],
                        axis=AX.X, op=OP.max,
                    )
                    for j in range(npair):
                        t = tp + j
                        h = hp.tile([P, K], bf16, tag="h")
                        nc.scalar.activation(
                            h[:], pa[:, j * K:(j + 1) * K], ACT.Sign,
                            bias=mxall[:, t:t + 1], scale=-1.0,
                        )
                        nc.tensor.matmul(
                            ps_stats[:], u_norm[:, t * 33:(t + 1) * 33], h[:],
                            start=(t == 0), stop=(t == nt - 1),
                        )
                return ps_stats

            def reduce_stats(ps_stats):
                """AllGather stats across cores + local sum -> S sbuf [33,K].

                Per-core stats fit int16 exactly (|v| <= 16384), halving the
                collective payload; the 8-way sum widens to f32 in one
                strided tensor_reduce.
                """
                i16 = mybir.dt.int16
                cc_in = dramp.tile([33, K], i16, tag="ccin")
                cc_out = dramp.tile([N_CORES * 33, K], i16, tag="ccout")
                S_loc = workp.tile([33, K], i16, tag="Sloc")
                nc.vector.tensor_copy(S_loc[:], ps_stats[:])
                nc.sync.dma_start(cc_in[:], S_loc[:])
                nc.gpsimd.collective_compute(
                    "AllGather", OP.bypass, replica_groups=rg,
                    ins=[cc_in.opt()], outs=[cc_out.opt()],
                )
                g8 = workp.tile([33, N_CORES * K], i16, tag="g8")
                nc.sync.dma_start(
                    g8[:].rearrange("j (r m) -> j r m", m=K),
                    cc_out[:].rearrange("(r j) m -> j r m", j=33),
                )
                S = workp.tile([33, K], f32, tag="S")
                nc.vector.tensor_reduce(
                    S[:], g8[:].rearrange("j (r m) -> j m r", m=K),
                    axis=AX.X, op=OP.add,
                )
                return S

            def make_W64(W_t):
                W64 = workp.tile([64 + CR, K], bf16, tag="W64")
                nc.sync.dma_start(W64[64:64 + CR, :], W_t[:])
                return W64

            for it in range(N_ITER):
                W64 = make_W64(W_cur)
                ps_stats = assign_pass(W_cur, W64)
                S = reduce_stats(ps_stats)

                # V = S - T = -true_stats
                V = workp.tile([33, K], f32, tag="V")
                nc.vector.tensor_scalar(V[:], S[:], T_sb[:], None,
                                        op0=OP.subtract)
                t01 = workp.tile([BITS, K], bf16, tag="t01")
                nc.vector.tensor_scalar(t01[:], V[0:BITS, :], 0.0, None,
                                        op0=OP.is_lt)
                # W_new rows = t01*2048 - 1024 in {-1024, +1024}
                W_next = workp.tile([CR, K], bf16, tag="W")
                nc.vector.tensor_scalar(W_next[0:BITS, :], t01[:], 2048.0,
                                        -1024.0, op0=OP.mult, op1=OP.add)
                nc.sync.dma_start(W_next[BITS:CR, :], wrows[:])
                # empty-cluster mask (count == 0 <=> V[32] == 0)
                cm = workp.tile([1, K], bf16, tag="cm")
                nc.vector.tensor_scalar(cm[:], V[32:33, :], 0.0, None,
                                        op0=OP.is_equal)
                pb = psS.tile([BITS, K], f32, tag="misc")
                nc.tensor.matmul(pb[:], ones_1x32[:], cm[:],
                                 start=True, stop=True)
                mask_u8 = workp.tile([BITS, K], u8, tag="mask")
                nc.vector.tensor_copy(mask_u8[:], pb[:])
                nc.vector.copy_predicated(W_next[0:BITS, :], mask_u8[:],
                                          W_cur[0:BITS, :])
                W_cur = W_next

            # ---------------- final assign + outputs ----------------
            W64 = make_W64(W_cur)
            ps_stats = assign_pass(W_cur, W64, save_mx=True)
            S = reduce_stats(ps_stats)

            # counts = -(S - T)[32]
            V = workp.tile([33, K], f32, tag="V")
            nc.vector.tensor_scalar(V[:], S[:], T_sb[:], None, op0=OP.subtract)
            cnt_f = workp.tile([1, K], f32, tag="cntf")
            nc.vector.tensor_scalar(cnt_f[:], V[32:33, :], -1.0, None,
                                    op0=OP.mult)
            cnt_i = workp.tile([1, K], i32, tag="cnti")
            nc.vector.tensor_copy(cnt_i[:], cnt_f[:])
            nc.sync.dma_start(counts_d[:], cnt_i[:])

            # labels: a = 511 - (mx & 1023) via integer ops (mod unsupported)
            lab_i32 = bigp.tile([P, nt], i32)
            nc.vector.tensor_copy(lab_i32[:], mxall[:])
            nc.vector.tensor_scalar(lab_i32[:], lab_i32[:], 1023, None,
                                    op0=OP.bitwise_and)
            lab_f = bigp.tile([P, nt], f32)
            nc.vector.tensor_copy(lab_f[:], lab_i32[:])
            nc.vector.tensor_scalar(lab_f[:], lab_f[:], -1.0, 511.0,
                                    op0=OP.mult, op1=OP.add)
            for t4 in range(0, nt, P):   # transpose nt columns in P-blocks
                w = min(P, nt - t4)
                ptr = psS.tile([P, P], f32, tag="misc")
                nc.tensor.transpose(ptr[0:w, 0:P], lab_f[:, t4:t4 + w],
                                    identf_sb[:])
                lab_i = workp.tile([P, P], i32, tag="labi")
                nc.vector.tensor_copy(lab_i[0:w, :], ptr[0:w, 0:P])
                nc.sync.dma_start(labels_d[t4:t4 + w, :], lab_i[0:w, :])

    nc.finalize()
    return nc


def _prep_inputs(x, planes):
    """Host-side sharding/layout prep (numpy only)."""
    import concourse.mybir as mybir
    bf = mybir.dt.np(mybir.dt.bfloat16)
    xT = np.ascontiguousarray(x.T, dtype=np.float32)          # [E, L]
    xinit_t = np.ascontiguousarray(x[::L // K].T, dtype=np.float32)  # [E, K]
    planes_t = np.ascontiguousarray(planes.T, dtype=np.float32)      # [E+1, 32]
    wrows = np.stack([_WROW_HI, _WROW_LO]).astype(bf)         # [2, K]
    ident_b = np.eye(P, dtype=np.float32).astype(bf)
    ident_f = np.eye(P, dtype=np.float32)
    in_maps = []
    for c in range(N_CORES):
        in_maps.append({
            "xt": np.ascontiguousarray(xT[:, c * LP:(c + 1) * LP]),
            "xinit_t": xinit_t,
            "planes_t": planes_t,
            "wrows": wrows,
            "ident_b": ident_b,
            "ident_f": ident_f,
        })
    return in_maps


_NC_CACHE = {}

LAST_EXEC_NS = None


def kernel(x, planes, k):
    import os
    from concourse.bass_utils import run_bass_kernel_spmd

    global LAST_EXEC_NS
    assert int(k) == K
    x = np.asarray(x, dtype=np.float32)
    planes = np.asarray(planes, dtype=np.float32)
    in_maps = _prep_inputs(x, planes)

    nt = LP // P
    if nt not in _NC_CACHE:
        _NC_CACHE[nt] = build_kernel(nt)
    nc = _NC_CACHE[nt]

    trace = bool(int(os.environ.get("BASS_KERNEL_TRACE", "0")))
    res = run_bass_kernel_spmd(nc, in_maps, core_ids=list(range(N_CORES)),
                               trace=trace)
    LAST_EXEC_NS = res.exec_time_ns
    outs = res.results
    labels = np.concatenate(
        [outs[c]["labels"].reshape(-1) for c in range(N_CORES)]
    ).astype(np.int32)
    counts = outs[0]["counts"].reshape(-1).astype(np.int32)
    return labels, counts
